# revision 12
# baseline (speedup 1.0000x reference)
# Trainium2 Bass kernel for windowed cross-attention (nn_CrossAttention).
#
# Reference computation (shapes hardcoded):
#   B=4, N=1024 (=32x32), C=512, NH=8 heads, HD=64
#   q = x_l @ Wq + bq    -> [B, NH, N, HD]   (query patch = whole image)
#   k = x_s @ Wk + bk    -> [B, NH, N, HD]   patchified 4x4 -> 64 patches x 16 tok
#   v = x_s @ Wv + bv
#   out[b,h,P,l,:] = softmax(q[b,h,l,:] @ k_patch[b,h,P].T * HD**-0.5) @ v_patch[b,h,P]
#   out shape [4, 8, 64, 1024, 64] fp32 (512 MB full) -> memory-bound on output writes
#
# Sharding: 8 cores = (B=4) x (head-half=2). Each core computes its b and 4
# heads. Device writes fp16 in a DMA-friendly contiguous layout (32 MB/core);
# the host reorders axes and upcasts to fp32 (pure layout transform).
#
# v3 design (PE row-group concurrency + engine balancing). The PE is SW/power
# throttled to ~1.2GHz on 8-core runs, so the kernel is engineered to maximize
# concurrent row-group streams on the PE and to balance the PSUM->SBUF staging
# load across ScalarE and VectorE:
#   - scores S^T[kt,l] per (u,g): the two l-halves run CONCURRENTLY in opposite
#     64-row groups (K/Q duplicated with swapped partition halves: KTb/QTb).
#   - exp on ScalarE -> E^T fp16 (scale folded, no max subtraction).
#   - per-patch sums: 2 matmuls/(u,g) with zero-padded [128,128] stationary so
#     lh0 denominators land in partitions 64..127 and lh1 in 0..63 of ONE
#     psum bank, accumulated over g; one DVE reciprocal per head.
#   - rcp broadcast to token rows (rcpB) via selA2 matmul placed in the row
#     half OPPOSITE to the out-matmul quadrants adjacent in issue order.
#   - normalize P^T = E^T * rcpB on DVE per l-half.
#   - out: 4 row-tiled (tile_position=(32j,0), K=32) matmuls per l-half;
#     j-pairs write the two banks of one [128,1024] psum pair-tile.
#   - staging PSUM->SBUF in [128,1024] ops split ScalarE/VectorE ~9:7;
#     one 1 MB contiguous DMA per (head, g) -> 32 MB/core.
#
# fp16 everywhere on-chip (values O(1)); measured rel err ~1e-3 vs float64.

import numpy as np

B, N, C = 4, 1024, 512
NH, HD = 8, 64
H4 = 4          # heads per core
NPATCH = 64     # 4x4 key/value patches
PTOK = 16       # tokens per patch
SCALE = HD ** -0.5

_CACHE = {}


def _token_perm():
    # natural token t = (4*tt+dy)*32 + 4*px + dx  ->  patch-major position
    # tt*128 + px*16 + dy*4 + dx   (patch P = tt*8+px, within-patch k = dy*4+dx)
    perm = np.empty(N, np.int64)
    for tt in range(8):
        for px in range(8):
            for dy in range(4):
                for dx in range(4):
                    perm[tt * 128 + px * 16 + dy * 4 + dx] = \
                        (4 * tt + dy) * 32 + 4 * px + dx
    return perm


def _build_program():
    import concourse.bass as bass
    import concourse.mybir as mybir
    from concourse import bacc
    from concourse.tile import TileContext

    f32 = mybir.dt.float32
    lp = mybir.dt.float16
    MULT = mybir.AluOpType.mult
    Exp = mybir.ActivationFunctionType.Exp
    Ident = mybir.ActivationFunctionType.Identity

    nc = bacc.Bacc("TRN2", target_bir_lowering=False, debug=False, num_devices=1)

    # x_s arrives token-permuted (patch-major) and fp16; x_l natural fp16.
    xl_d = nc.dram_tensor("xl", [N, C], lp, kind="ExternalInput")
    xs_d = nc.dram_tensor("xs", [N, C], lp, kind="ExternalInput")
    wq_d = nc.dram_tensor("wq", [C, 256], lp, kind="ExternalInput")
    wk_d = nc.dram_tensor("wk", [C, 256], lp, kind="ExternalInput")
    wv_d = nc.dram_tensor("wv", [C, 256], lp, kind="ExternalInput")
    bq_d = nc.dram_tensor("bq2", [128, 2], f32, kind="ExternalInput")
    bk_d = nc.dram_tensor("bk2", [128, 2], f32, kind="ExternalInput")
    bv_d = nc.dram_tensor("bv_row", [1, 256], lp, kind="ExternalInput")
    one_d = nc.dram_tensor("ones_row", [1, 128], lp, kind="ExternalInput")
    msk_d = nc.dram_tensor("maskp", [128, 128], lp, kind="ExternalInput")
    onesS_d = nc.dram_tensor("onesS", [128, 2048], lp, kind="ExternalInput")
    selA2_d = nc.dram_tensor("selA2", [128, 1024], lp, kind="ExternalInput")
    out_d = nc.dram_tensor("out_c", [H4, 8, 128, 4096], lp, kind="ExternalOutput")

    with TileContext(nc) as tc:
        with (
            tc.tile_pool(name="singles", bufs=1) as sg,
            tc.tile_pool(name="vbd", bufs=32) as vbd_p,
            tc.tile_pool(name="et", bufs=2) as et_p,
            tc.tile_pool(name="pt", bufs=6) as pt_p,
            tc.tile_pool(name="rc", bufs=2) as rc_p,
            tc.tile_pool(name="stage", bufs=8) as st_p,
            # PSUM budget (8 banks of 2KB/partition):
            #   psA  1 x [128,1024]f32 -> 2 banks (scores S^T)
            #   psS  1 x [128, 512]f32 -> 1 bank  (packed lh sums, acc over g)
            #   psP  2 x [128,1024]f32 -> 4 banks (out pair-tiles, j01/j23)
            #   psR  1 x [128, 512]f32 -> 1 bank  (rcpB ring)
            tc.tile_pool(name="psA", bufs=1, space="PSUM") as psA,
            tc.tile_pool(name="psS", bufs=1, space="PSUM") as psS,
            tc.tile_pool(name="psP", bufs=2, space="PSUM") as psP,
            tc.tile_pool(name="psR", bufs=1, space="PSUM") as psR,
        ):
            # ---- constants / inputs (x_s path first: V+K gate first output) --
            xsT = sg.tile([128, 4, N], lp, name="xsT")   # [c_lo, ko, tok(perm)]
            for ko in range(4):
                nc.sync.dma_start(xsT[:, ko, :], xs_d.ap()[:, ko * 128:(ko + 1) * 128],
                                  transpose=True)
            wv = sg.tile([128, 4, 256], lp, name="wv_t")
            nc.sync.dma_start(wv[:], wv_d.ap().rearrange("(ko ki) m -> ki ko m", ki=128))
            wk = sg.tile([128, 4, 256], lp, name="wk_t")
            nc.sync.dma_start(wk[:], wk_d.ap().rearrange("(ko ki) m -> ki ko m", ki=128))
            bvr = sg.tile([1, 256], lp, name="bvr_t")
            nc.sync.dma_start(bvr[:], bv_d.ap())
            ones = sg.tile([1, 128], lp, name="ones_t")
            nc.sync.dma_start(ones[:], one_d.ap())
            msk = sg.tile([128, 128], lp, name="msk_t")
            nc.sync.dma_start(msk[:], msk_d.ap())
            onesS = sg.tile([128, 2, 8, 128], lp, name="onesS_t")
            nc.sync.dma_start(onesS[:], onesS_d.ap().rearrange(
                "p (lh g m) -> p lh g m", lh=2, g=8))
            selA2 = sg.tile([128, 1024], lp, name="selA2_t")
            nc.sync.dma_start(selA2[:], selA2_d.ap())
            xlT = sg.tile([128, 4, N], lp, name="xlT")   # [c_lo, ko, token]
            for ko in range(4):
                nc.sync.dma_start(xlT[:, ko, :], xl_d.ap()[:, ko * 128:(ko + 1) * 128],
                                  transpose=True)
            wq = sg.tile([128, 4, 256], lp, name="wq_t")
            nc.sync.dma_start(wq[:], wq_d.ap().rearrange("(ko ki) m -> ki ko m", ki=128))
            bq2 = sg.tile([128, 2], f32, name="bq2_t")
            bk2 = sg.tile([128, 2], f32, name="bk2_t")
            nc.sync.dma_start(bq2[:], bq_d.ap())
            nc.sync.dma_start(bk2[:], bk_d.ap())

            QT = sg.tile([128, 2, N], lp, name="QT")     # [outC_lo, tile, token]
            KT = sg.tile([128, 2, N], lp, name="KT")     # tokens patch-permuted
            QTb = sg.tile([128, 2, N], lp, name="QTb")   # partition halves swapped
            KTb = sg.tile([128, 2, N], lp, name="KTb")
            vperm = sg.tile([128, 8, 256], lp, name="vperm")  # [tok(perm), tt, outC]

            # ---- V projection (tokens on partitions, patch order) ----
            for tt in range(8):
                vp = psP.tile([128, 512], f32, tag="opair", name="vp")
                for ko in range(4):
                    nc.tensor.matmul(vp[:, :256], xsT[:, ko, tt * 128:(tt + 1) * 128],
                                     wv[:, ko, :], start=(ko == 0), stop=False)
                nc.tensor.matmul(vp[:, :256], ones[:, :], bvr[:],
                                 start=False, stop=True)
                nc.vector.tensor_copy(vperm[:, tt, :], vp[:, :256])

            # ---- K/Q projections: [outC, token] = W.T @ x.T, bias on ScalarE --
            for wt, xt, dst, dstb, bias in ((wk, xsT, KT, KTb, bk2),
                                            (wq, xlT, QT, QTb, bq2)):
                for m in range(2):
                    for n in range(2):
                        pp = psP.tile([128, 512], f32, tag="opair", name="pp")
                        for ko in range(4):
                            nc.tensor.matmul(
                                pp[:],
                                wt[:, ko, m * 128:(m + 1) * 128],
                                xt[:, ko, n * 512:(n + 1) * 512],
                                start=(ko == 0), stop=(ko == 3))
                        nc.scalar.activation(dst[:, m, n * 512:(n + 1) * 512], pp[:],
                                             Ident, bias=bias[:, m:m + 1], scale=1.0)
                    # swapped-half duplicate for this m-tile so score matmuls
                    # can run in either 64-row group (DMA crosses partitions)
                    nc.sync.dma_start(dstb[0:64, m, :], dst[64:128, m, :])
                    nc.sync.dma_start(dstb[64:128, m, :], dst[0:64, m, :])

            # ---- masked V pair-blocks: Vbd[u*8+g][t, pp*64+hd] =
            #      ((t//16)%2 == pp) * vperm[t, g, u*64+hd] ----
            vbd = {}

            def emit_vbd(u, g):
                t = vbd_p.tile([128, 128], lp, tag="vbd", name="vbdt")
                nc.vector.tensor_tensor(
                    t.rearrange("p (pp hd) -> p pp hd", pp=2),
                    msk.rearrange("p (pp hd) -> p pp hd", pp=2),
                    vperm[:, g, u * 64:(u + 1) * 64][:, None, :].to_broadcast(
                        (128, 2, 64)),
                    MULT)
                vbd[(u, g)] = t

            for g in range(8):
                emit_vbd(0, g)

            # ---- main attention loop (software-pipelined over u = head) ----
            ET = [None, None]
            sums = [None]
            rcp16 = [None]

            def emit_B1_sc(u, g):
                # scores + exp for head u, token-group g
                th, po = u // 2, (u % 2) * 64
                if g == 0:
                    ET[u % 2] = et_p.tile([128, 8, N], lp, tag="ET", name=f"ET{u}")
                # two single-bank score tiles + two exp ops: the serial
                # scores->exp->scores chain pipelines per l-half instead of
                # serializing on one 2-bank tile.
                sTa = psA.tile([128, 512], f32, tag="sta", name="sTa")
                sTb = psA.tile([128, 512], f32, tag="stb", name="sTb")
                # lh=0 in rows po..po+63 (KT/QT), lh=1 in the opposite half
                # (KTb/QTb hold the same data with partition halves swapped)
                # -> the two 512-col streams run in disjoint PE row groups.
                nc.tensor.matmul(sTa[:],
                                 KT[po:po + 64, th, g * 128:(g + 1) * 128],
                                 QT[po:po + 64, th, 0:512],
                                 start=True, stop=True)
                po2 = po ^ 64
                nc.tensor.matmul(sTb[:],
                                 KTb[po2:po2 + 64, th, g * 128:(g + 1) * 128],
                                 QTb[po2:po2 + 64, th, 512:1024],
                                 start=True, stop=True)
                nc.scalar.activation(ET[u % 2][:, g, 0:512], sTa[:], Exp, scale=SCALE)
                nc.scalar.activation(ET[u % 2][:, g, 512:1024], sTb[:], Exp,
                                     scale=SCALE)

            def emit_B1_su(u, g):
                # packed sums: one bank, lh0 -> partitions 64..127, lh1 -> 0..63
                # (zero-padded stationaries overlap-accumulate harmlessly).
                # Emitted one slot AFTER its exp so the sums matmul never
                # blocks the PE FIFO waiting on ScalarE.
                if g == 0:
                    sums[0] = psS.tile([128, 512], f32, tag="sums", name="sums")
                for lh in range(2):
                    nc.tensor.matmul(
                        sums[0][:],
                        onesS[:, lh, g, :],
                        ET[u % 2][:, g, lh * 512:(lh + 1) * 512],
                        start=(g == 0 and lh == 0), stop=(g == 7 and lh == 1))

            def emit_rcp(u):
                rcp32 = rc_p.tile([128, 512], f32, tag="rcp32", name=f"rcp32_{u}")
                nc.vector.reciprocal_approx_fast(rcp32[:], sums[0][:])
                rcp16[0] = rc_p.tile([128, 512], lp, tag="rcp16", name=f"rcp16_{u}")
                nc.scalar.copy(rcp16[0][:], rcp32[:])

            PTs = {}

            def emit_rcpB_PT(u, g, lh):
                # one g-group AHEAD of emit_out: the output matmuls never wait
                # on the DVE normalize. rcpB(lh0) runs in rows 64..127 (and
                # lh1 in 0..63), i.e. OPPOSITE to nothing in particular but
                # staggered against the out quadrants adjacent in the FIFO.
                if lh == 0:
                    PTs[(u, g)] = pt_p.tile([128, 1024], lp, tag="PT", name="PT")
                PT = PTs[(u, g)]
                ls = slice(lh * 512, (lh + 1) * 512)
                ro = 64 - lh * 64          # lh0 -> rows 64.., lh1 -> rows 0..
                rcpB = psR.tile([128, 512], f32, tag="rcpB", name="rcpB")
                nc.tensor.matmul(rcpB[:], selA2[ro:ro + 64, g * 128:(g + 1) * 128],
                                 rcp16[0][ro:ro + 64, :], start=True, stop=True)
                nc.vector.tensor_tensor(PT[:, ls], ET[u % 2][:, g, ls],
                                        rcpB[:], MULT)

            # staging schedule: ~9/16 of the [128,1024] PSUM->SBUF staging ops
            # go to ScalarE, 7/16 to VectorE (balances exp vs PT-normalize).
            def stage_engine(idx):
                return ((idx * 9) // 16) != (((idx + 1) * 9) // 16)  # True->Scalar

            stages = {}

            def emit_out_lh(u, g, lh):
                PT = PTs[(u, g)]
                if lh == 0:
                    stages[(u, g)] = st_p.tile([128, 4096], lp, tag="stage", name="stage")
                stage = stages[(u, g)]
                ls = slice(lh * 512, (lh + 1) * 512)
                for jp in range(2):          # quadrant pairs (j01, j23)
                    pr = psP.tile([128, 1024], f32, tag="opair", name="pr")
                    for jj in range(2):
                        j = jp * 2 + jj
                        nc.tensor.matmul(pr[:, jj * 512:(jj + 1) * 512],
                                         vbd[(u, g)][32 * j:32 * j + 32, :],
                                         PT[32 * j:32 * j + 32, ls],
                                         start=True, stop=True,
                                         tile_position=(32 * j, 0))
                    # stage layout: free = (lh, jp, jj, l) -> contiguous 1024
                    dst = stage[:, lh * 2048 + jp * 1024: lh * 2048 + (jp + 1) * 1024]
                    idx = ((u * 8 + g) * 2 + lh) * 2 + jp
                    if stage_engine(idx):
                        nc.scalar.copy(dst, pr[:])
                    else:
                        nc.vector.tensor_copy(dst, pr[:])

            def emit_dma_half(u, g, lh):
                st = stages[(u, g)]
                eng = nc.sync if lh == 0 else nc.gpsimd
                eng.dma_start(out_d.ap()[u][g][:, lh * 2048:(lh + 1) * 2048],
                              st[:, lh * 2048:(lh + 1) * 2048])
                if lh == 1:
                    stages.pop((u, g))

            for g in range(8):
                emit_B1_sc(0, g)
                emit_B1_su(0, g)
            for u in range(H4):
                emit_rcp(u)
                emit_rcpB_PT(u, 0, 0)
                emit_rcpB_PT(u, 0, 1)
                for g in range(8):
                    if u + 1 < H4:
                        emit_B1_sc(u + 1, g)
                        if g > 0:
                            emit_B1_su(u + 1, g - 1)
                        emit_vbd(u + 1, g)
                    if g + 1 < 8:
                        emit_rcpB_PT(u, g + 1, 0)
                    emit_out_lh(u, g, 0)
                    emit_dma_half(u, g, 0)
                    if g + 1 < 8:
                        emit_rcpB_PT(u, g + 1, 1)
                    emit_out_lh(u, g, 1)
                    emit_dma_half(u, g, 1)
                if u + 1 < H4:
                    emit_B1_su(u + 1, 7)

    nc.compile()
    return nc


def _host_inputs(x_l, x_s, Wq, bq, Wk, bk, Wv, bv):
    f16 = np.float16
    perm = _token_perm()
    # maskp[t, pp*64+hd] = ((t//16)%2 == pp)
    tt16 = (np.arange(128) // 16) % 2
    maskp = (tt16[:, None] == (np.arange(128) // 64)[None, :]).astype(f16)
    # onesS[t, lh*1024 + g*128 + (64 - lh*64 + g*8 + t//16)] = 1
    #   -> sums matmul stationary [128,128] per (lh, g): col = out partition.
    #      lh0 denominators land in partitions 64..127, lh1 in 0..63.
    t16 = np.arange(128) // 16
    onesS = np.zeros((128, 2, 8, 128), f16)
    for lh in range(2):
        for g in range(8):
            for t in range(128):
                onesS[t, lh, g, (64 - lh * 64) + g * 8 + t16[t]] = 1.0
    # selA2[ro + r, g*128 + kt] = (r == g*8 + kt//16), for ro in {0, 64}
    selA2 = np.zeros((128, 1024), f16)
    for g in range(8):
        for kt in range(128):
            selA2[g * 8 + kt // 16, g * 128 + kt] = 1.0
            selA2[64 + g * 8 + kt // 16, g * 128 + kt] = 1.0
    ones_row = np.ones((1, 128), f16)
    in_maps = []
    for core in range(8):
        b, hh = core // 2, core % 2
        cs = slice(hh * 256, (hh + 1) * 256)
        in_maps.append({
            "xl": np.ascontiguousarray(x_l[b].astype(f16)),
            "xs": np.ascontiguousarray(x_s[b][perm].astype(f16)),
            "wq": np.ascontiguousarray(Wq[:, cs].astype(f16)),
            "wk": np.ascontiguousarray(Wk[:, cs].astype(f16)),
            "wv": np.ascontiguousarray(Wv[:, cs].astype(f16)),
            "bq2": np.ascontiguousarray(bq[cs].reshape(2, 128).T.astype(np.float32)),
            "bk2": np.ascontiguousarray(bk[cs].reshape(2, 128).T.astype(np.float32)),
            "bv_row": bv[cs].reshape(1, 256).astype(f16),
            "ones_row": ones_row,
            "maskp": maskp,
            "onesS": np.ascontiguousarray(onesS.reshape(128, 2048)),
            "selA2": selA2,
        })
    return in_maps


def _run(in_maps, trace=False):
    from concourse.bass_utils import run_bass_kernel_spmd
    if "prog" not in _CACHE:
        _CACHE["prog"] = _build_program()
    nc = _CACHE["prog"]
    res = run_bass_kernel_spmd(nc, in_maps, core_ids=list(range(8)), trace=trace)
    return res


def kernel(x_s, x_l, Wq, bq, Wk, bk, Wv, bv, H=None, W=None, **_unused):
    in_maps = _host_inputs(np.asarray(x_l, np.float32), np.asarray(x_s, np.float32),
                           np.asarray(Wq, np.float32), np.asarray(bq, np.float32),
                           np.asarray(Wk, np.float32), np.asarray(bk, np.float32),
                           np.asarray(Wv, np.float32), np.asarray(bv, np.float32))
    res = _run(in_maps)
    out = np.empty((B, NH, NPATCH, N, HD), np.float32)
    for core in range(8):
        b, hh = core // 2, core % 2
        # device layout: [u(head), g, (pp,hd) partitions, (lh, j, l) free] fp16
        arr = np.asarray(res.results[core]["out_c"]).reshape(
            H4, 8, 2, 64, 2, 4, 512)
        # [u, g, pp, hd, lh, j, l'] -> [u, g, j, pp, (lh,l'), hd]
        #   P = g*8 + 2*j + pp, l = lh*512 + l'
        out[b, hh * 4:(hh + 1) * 4] = arr.transpose(0, 1, 5, 2, 4, 6, 3).reshape(
            H4, NPATCH, N, HD)
    return out
